# revision 1
# baseline (speedup 1.0000x reference)
"""BERT-base (12-layer, B=8, T=512, D=768) forward + tied-embedding LM head
on 8 Trainium2 NeuronCores.

Sharding: data-parallel over the batch dimension -- core b computes batch
element b end-to-end (no collectives). Activations are kept feature-major
[D, T] in SBUF so every GEMM consumes weights in their natural [d_in, d_out]
layout; attention scores are computed key-major so softmax reductions become
ones-matmuls / an appended ones-column on V; the LM head runs token-major so
logits come out [T, V] without any transposes. All GEMMs run in bf16 with
fp32 PSUM accumulation; the residual stream stays fp32.
"""

from contextlib import ExitStack

import numpy as np
import ml_dtypes

import concourse.bass as bass
import concourse.bacc as bacc
import concourse.mybir as mybir
import concourse.tile as tile
from concourse import bass_utils
from concourse._compat import get_trn_type

NP_BF16 = ml_dtypes.bfloat16

FP32 = mybir.dt.float32
BF16 = mybir.dt.bfloat16
AF = mybir.ActivationFunctionType
OP = mybir.AluOpType

P = 128
T = 512
D = 768
H = 12
HD = 64
DF = 3072
V = 30522
DK = D // P      # 6 contraction chunks over D
TCH = T // P     # 4 token chunks
FK = DF // P     # 24 contraction chunks over DF
SCALE = 0.125    # 1/sqrt(64)
EPS = 1e-5


def bcast_ap(t, nparts, free):
    """partition-broadcast view of a [1, free] sbuf tile -> [nparts, free]"""
    return bass.AP(tensor=t.tensor, offset=t.offset, ap=[[0, nparts], [1, free]])


def build(n_layers=12, with_head=True, debug_taps=()):
    nc = bacc.Bacc(get_trn_type() or "TRN2", target_bir_lowering=False, debug=False)

    x0T = nc.dram_tensor("x0T", [D, T], FP32, kind="ExternalInput")
    mb = nc.dram_tensor("mb", [P, TCH], FP32, kind="ExternalInput")
    L = max(n_layers, 1)
    wqk = nc.dram_tensor("wqk", [L, D, 2 * D], BF16, kind="ExternalInput")
    wv = nc.dram_tensor("wv", [L, D, D], BF16, kind="ExternalInput")
    wpr = nc.dram_tensor("wpr", [L, D, D], BF16, kind="ExternalInput")
    wfc = nc.dram_tensor("wfc", [L, D, DF], BF16, kind="ExternalInput")
    wf2 = nc.dram_tensor("wf2", [L, DF, D], BF16, kind="ExternalInput")
    if with_head:
        wembT = nc.dram_tensor("wembT", [D, V], BF16, kind="ExternalInput")
        out = nc.dram_tensor("out", [T, V], FP32, kind="ExternalOutput")
    else:
        out = nc.dram_tensor("out", [D, T], FP32, kind="ExternalOutput")

    tap_specs = {
        "h1": ([D, T], BF16), "qkT": ([2 * D, T], BF16),
        "v65": ([TCH * P, H * (HD + 1)], BF16), "p0": ([TCH * P, T], BF16),
        "yT": ([D, T], BF16), "xattn": ([D, T], FP32), "h2": ([D, T], BF16),
        "gT": ([DF, T], BF16), "xfinal": ([D, T], FP32),
        "yraw": ([H * (HD + 1), T], FP32), "invb": ([H, T], FP32),
    }
    taps = {}
    for name in debug_taps:
        shape, dt = tap_specs[name]
        taps[name] = nc.dram_tensor(f"tap_{name}", shape, dt, kind="ExternalOutput")

    with tile.TileContext(nc) as tc, ExitStack() as ctx:
        consts = ctx.enter_context(tc.tile_pool(name="consts", bufs=1))
        resid = ctx.enter_context(tc.tile_pool(name="resid", bufs=1))
        acts = ctx.enter_context(tc.tile_pool(name="acts", bufs=2))
        wpool = ctx.enter_context(tc.tile_pool(name="wpool", bufs=1))
        small = ctx.enter_context(tc.tile_pool(name="small", bufs=1))
        ps_stat = ctx.enter_context(tc.tile_pool(name="ps_stat", bufs=2, space="PSUM"))
        ps_gemm = ctx.enter_context(tc.tile_pool(name="ps_gemm", bufs=4, space="PSUM"))
        ps_av = ctx.enter_context(tc.tile_pool(name="ps_av", bufs=2, space="PSUM"))
        dscratch = ctx.enter_context(tc.tile_pool(name="dscratch", bufs=4, space="DRAM"))

        czero = consts.tile([P, 1], FP32, tag="czero")
        nc.vector.memset(czero[:], 0.0)
        ceps = consts.tile([P, 1], FP32, tag="ceps")
        nc.vector.memset(ceps[:], EPS)
        nc.const_aps.aps[(FP32, 0.0)] = czero[:]
        nc.const_aps.aps[(FP32, EPS)] = ceps[:]

        ones_f32 = consts.tile([P, 1], FP32, tag="ones_f32")
        nc.vector.memset(ones_f32[:], 1.0)
        ones_bf = consts.tile([P, 1], BF16, tag="ones_bf")
        nc.vector.memset(ones_bf[:], 1.0)
        ones_row = consts.tile([1, P], FP32, tag="ones_row")
        nc.vector.memset(ones_row[:], 1.0)
        mb_sb = consts.tile([P, TCH], FP32, tag="mb_sb")
        nc.sync.dma_start(mb_sb[:], mb[:])

        # residual stream
        xT = []
        for j in range(DK):
            t = resid.tile([P, T], FP32, tag=f"x{j}")
            nc.sync.dma_start(t[:], x0T[j * P:(j + 1) * P, :])
            xT.append(t)

        def layer_norm(tag):
            """feature-major LN over xT -> 6 bf16 tiles"""
            sum_ps = ps_stat.tile([1, T], FP32, tag="stat")
            ssq_ps = ps_stat.tile([1, T], FP32, tag="stat")
            sq_tiles = []
            for j in range(DK):
                sq = acts.tile([P, T], BF16, tag="sq", bufs=6)
                nc.scalar.activation(sq[:], xT[j][:], AF.Square)
                sq_tiles.append(sq)
                nc.tensor.matmul(sum_ps[:], ones_f32[:], xT[j][:],
                                 start=(j == 0), stop=(j == DK - 1))
            for j in range(DK):
                nc.tensor.matmul(ssq_ps[:], ones_bf[:], sq_tiles[j][:],
                                 start=(j == 0), stop=(j == DK - 1))
            nm = small.tile([1, T], FP32, tag="nm")
            nc.vector.tensor_scalar_mul(nm[:], sum_ps[:], -1.0 / D)
            msq = small.tile([1, T], FP32, tag="msq")
            nc.vector.tensor_mul(msq[:], nm[:], nm[:])
            var = small.tile([1, T], FP32, tag="var")
            nc.vector.scalar_tensor_tensor(
                out=var[:], in0=ssq_ps[:], scalar=1.0 / D, in1=msq[:],
                op0=OP.mult, op1=OP.subtract)
            lnv = small.tile([1, T], FP32, tag="lnv")
            nc.scalar.activation(lnv[:], var[:], AF.Ln, bias=EPS)
            rstd = small.tile([1, T], FP32, tag="rstd")
            nc.scalar.activation(rstd[:], lnv[:], AF.Exp, scale=-0.5)
            nmrs = small.tile([1, T], FP32, tag="nmrs")  # -mean*rstd
            nc.vector.tensor_mul(nmrs[:], nm[:], rstd[:])
            # broadcast rstd and -mean*rstd across partitions via ones-matmul
            rstd_b = ps_stat.tile([P, T], FP32, tag="stat", name="rstd_b")
            nc.tensor.matmul(rstd_b[:], ones_row[:], rstd[:], start=True, stop=True)
            nmrs_b = ps_stat.tile([P, T], FP32, tag="stat", name="nmrs_b")
            nc.tensor.matmul(nmrs_b[:], ones_row[:], nmrs[:], start=True, stop=True)
            h_tiles = []
            for j in range(DK):
                h = acts.tile([P, T], BF16, tag=f"h_{tag}", bufs=6)
                # h = x*rstd_b + nmrs_b
                nc.vector.tensor_mul(h[:], xT[j][:], rstd_b[:])
                nc.vector.tensor_add(h[:], h[:], nmrs_b[:])
                h_tiles.append(h)
            return h_tiles

        def gemm_fm(w3, l, M, rhs_tiles, tag, CG, evac, nk=DK):
            """feature-major GEMM: for each m-chunk of M, psum[128,T] =
            sum_k w3[l, k*128:(k+1)*128, m-chunk].T @ rhs_tiles[k]; column
            groups of CG limit slab residency."""
            for cg0 in range(0, M, CG):
                cgn = min(CG, M - cg0)
                slabs = []
                for k in range(nk):
                    s = wpool.tile([P, CG], BF16, tag=f"{tag}_{k}", bufs=2)
                    nc.sync.dma_start(s[:, :cgn], w3[l, k * P:(k + 1) * P, cg0:cg0 + cgn])
                    slabs.append(s)
                for mi in range(cgn // P):
                    m = (cg0 // P) + mi
                    ps = ps_gemm.tile([P, T], FP32, tag="g")
                    for k in range(nk):
                        nc.tensor.matmul(ps[:], slabs[k][:, mi * P:(mi + 1) * P],
                                         rhs_tiles[k][:],
                                         start=(k == 0), stop=(k == nk - 1))
                    evac(m, ps)

        def dump_tiles(name, tiles, rows=P):
            if name in taps:
                for j, t in enumerate(tiles):
                    nc.sync.dma_start(taps[name][j * rows:(j + 1) * rows, :], t[:])

        def layer(l):
            h1 = layer_norm("ln1")
            dump_tiles("h1", h1)

            # ---- QK gemm (feature-major): qkT[c,t], c in [0,1536) ----
            qkT = [None] * (2 * D // P)

            def qk_evac(m, ps):
                qt = acts.tile([P, T], BF16, tag="qkT", bufs=12)
                nc.vector.tensor_copy(qt[:], ps[:])
                qkT[m] = qt
            gemm_fm(wqk, l, 2 * D, h1, "wqk", T, qk_evac)
            dump_tiles("qkT", qkT)

            # ---- V gemm (token-major): v[t, c] with ones column per head ----
            v_slabs = []
            for k in range(DK):
                s = wpool.tile([P, D], BF16, tag=f"wv_{k}", bufs=1)
                nc.sync.dma_start(s[:], wv[l, k * P:(k + 1) * P, :])
                v_slabs.append(s)
            v65 = []
            for tch in range(TCH):
                vt = acts.tile([P, H, HD + 1], BF16, tag="v65", bufs=5)
                nc.vector.memset(vt[:, :, HD:HD + 1], 1.0)
                for n0 in range(0, D, T):  # n in {0, 512} sizes {512, 256}
                    nn = min(T, D - n0)
                    ps = ps_gemm.tile([P, T], FP32, tag="g")
                    for k in range(DK):
                        nc.tensor.matmul(
                            ps[:, :nn],
                            h1[k][:, tch * P:(tch + 1) * P],
                            v_slabs[k][:, n0:n0 + nn],
                            start=(k == 0), stop=(k == DK - 1))
                    dst = vt[:, n0 // HD:(n0 + nn) // HD, 0:HD]
                    src = ps[:, :nn].rearrange("p (h d) -> p h d", d=HD)
                    nc.vector.tensor_copy(dst, src)
                v65.append(vt)
            if "v65" in taps:
                for j, t in enumerate(v65):
                    nc.sync.dma_start(
                        taps["v65"][j * P:(j + 1) * P, :],
                        t[:].rearrange("p h d -> p (h d)"))

            # ---- attention per head ----
            yT = [acts.tile([P, T], BF16, tag="yT", bufs=6, name=f"yT{i}")
                  for i in range(DK)]
            for h in range(H):
                ht, r = h // 2, h % 2
                qt = qkT[ht]
                kt = qkT[DK + ht]
                rows = slice(r * HD, (r + 1) * HD)
                p_tiles = []
                for kc in range(TCH):
                    s_ps = ps_gemm.tile([P, T], FP32, tag="g")
                    nc.tensor.matmul(s_ps[:], kt[rows, kc * P:(kc + 1) * P],
                                     qt[rows, :], start=True, stop=True)
                    pt = acts.tile([P, T], BF16, tag="p", bufs=5)
                    nc.scalar.activation(pt[:], s_ps[:], AF.Exp,
                                         bias=mb_sb[:, kc:kc + 1], scale=SCALE)
                    p_tiles.append(pt)
                if h == 0 and "p0" in taps:
                    for kc in range(TCH):
                        nc.sync.dma_start(
                            taps["p0"][kc * P:(kc + 1) * P, :], p_tiles[kc][:])
                y_ps = ps_av.tile([HD + 1, T], FP32, tag="av")
                for kc in range(TCH):
                    nc.tensor.matmul(y_ps[:], v65[kc][:, h, :], p_tiles[kc][:],
                                     start=(kc == 0), stop=(kc == TCH - 1))
                if "yraw" in taps:
                    yr = acts.tile([HD + 1, T], FP32, tag="yraw", bufs=2)
                    nc.vector.tensor_copy(yr[:], y_ps[:])
                    nc.sync.dma_start(
                        taps["yraw"][h * (HD + 1):(h + 1) * (HD + 1), :], yr[:])
                # reciprocal of the sum row: lanes are partition-locked, so
                # stage at partition HD, round-trip through DRAM to broadcast
                # 1/sum = exp(-ln(sum)) on ACT (same table set as softmax exp;
                # reciprocal_approx_fast only works at partition base 0)
                lns = acts.tile([HD + 1, T], BF16, tag="lns", bufs=2)
                nc.scalar.activation(lns[HD:HD + 1, :], y_ps[HD:HD + 1, :], AF.Ln)
                inv_st = acts.tile([HD + 1, T], BF16, tag="inv_st", bufs=2)
                nc.scalar.activation(inv_st[HD:HD + 1, :], lns[HD:HD + 1, :],
                                     AF.Exp, scale=-1.0)
                invd = dscratch.tile([1, T], BF16, tag="invd", bufs=4)
                nc.sync.dma_start(invd[:], inv_st[HD:HD + 1, :])
                inv_b = acts.tile([HD, T], BF16, tag="inv_b", bufs=2)
                nc.sync.dma_start(inv_b[:], bcast_ap(invd, HD, T))
                if "invb" in taps:
                    nc.sync.dma_start(taps["invb"][h:h + 1, :], inv_st[HD:HD + 1, :])
                if r == 0:
                    nc.vector.tensor_mul(yT[ht][rows, :], y_ps[0:HD, :], inv_b[:])
                else:
                    ytmp = acts.tile([HD, T], BF16, tag="ytmp", bufs=2)
                    nc.vector.tensor_mul(ytmp[:], y_ps[0:HD, :], inv_b[:])
                    nc.sync.dma_start(yT[ht][rows, :], ytmp[:])

            dump_tiles("yT", yT)

            # ---- proj gemm + residual ----
            def resid_evac(m, ps):
                nc.vector.tensor_add(xT[m][:], xT[m][:], ps[:])
            gemm_fm(wpr, l, D, yT, "wpr", 3 * P, resid_evac)

            dump_tiles("xattn", xT)
            h2 = layer_norm("ln2")
            dump_tiles("h2", h2)

            # ---- fc1 gemm + gelu ----
            gT = [None] * FK

            def gelu_evac(m, ps):
                g = acts.tile([P, T], BF16, tag="gT", bufs=24)
                nc.scalar.activation(g[:], ps[:], AF.Gelu_apprx_tanh)
                gT[m] = g
            gemm_fm(wfc, l, DF, h2, "wfc", 2 * 3 * P, gelu_evac)

            dump_tiles("gT", gT)

            # ---- fc2 gemm + residual ----
            gemm_fm(wf2, l, D, gT, "wf2", P, resid_evac, nk=FK)

        for l in range(n_layers):
            layer(l)

        if not with_head:
            for j in range(DK):
                nc.sync.dma_start(out[j * P:(j + 1) * P, :], xT[j][:])
        else:
            # ---- LM head: logits[t, v] = x @ wembT ----
            xbf = []
            for j in range(DK):
                xb = acts.tile([P, T], BF16, tag="xbf", bufs=6)
                nc.vector.tensor_copy(xb[:], xT[j][:])
                xbf.append(xb)
            wT3 = wembT.rearrange("(ko ki) v -> ki ko v", ki=P)
            NV = 512
            for vs in range(0, V, NV):
                nn = min(NV, V - vs)
                w_sb = wpool.tile([P, DK, NV], BF16, tag="whead", bufs=2)
                nc.sync.dma_start(w_sb[:, :, :nn], wT3[:, :, vs:vs + nn])
                for tch in range(TCH):
                    ps = ps_gemm.tile([P, NV], FP32, tag="g")
                    for k in range(DK):
                        nc.tensor.matmul(
                            ps[:, :nn], xbf[k][:, tch * P:(tch + 1) * P],
                            w_sb[:, k, :nn], start=(k == 0), stop=(k == DK - 1))
                    o = acts.tile([P, NV], FP32, tag="o_head", bufs=3)
                    if tch % 2 == 0:
                        nc.vector.tensor_copy(o[:, :nn], ps[:, :nn])
                    else:
                        nc.scalar.copy(o[:, :nn], ps[:, :nn])
                    nc.sync.dma_start(out[tch * P:(tch + 1) * P, vs:vs + nn], o[:, :nn])

    nc.compile()
    return nc


# ---------------------------------------------------------------------------
# host side
# ---------------------------------------------------------------------------

B = 8
NCORES = 8


def _np_layer_norm(x, g, b, eps=1e-5):
    m = x.mean(-1, keepdims=True)
    v = x.var(-1, keepdims=True)
    return (x - m) / np.sqrt(v + eps) * g + b


def _prep_in_maps(inputs):
    ids = np.asarray(inputs["input_ids"]).astype(np.int64)
    tt = np.asarray(inputs["token_type_ids"]).astype(np.int64)
    x0 = (np.asarray(inputs["word_emb"], np.float32)[ids]
          + np.asarray(inputs["pos_emb"], np.float32)[None, :ids.shape[1], :]
          + np.asarray(inputs["type_emb"], np.float32)[tt])
    x0 = _np_layer_norm(x0, np.asarray(inputs["emb_ln_g"], np.float32),
                        np.asarray(inputs["emb_ln_b"], np.float32))
    mask = np.asarray(inputs["attention_mask"], np.float32)

    wqkv = np.asarray(inputs["wqkv"], np.float32)
    wfc_in = np.asarray(inputs["wfc"], np.float32)
    ln1_g = np.asarray(inputs["ln1_g"], np.float32)
    ln2_g = np.asarray(inputs["ln2_g"], np.float32)
    for name in ("bqkv", "bproj", "bfc", "bfc2", "ln1_b", "ln2_b"):
        assert np.abs(np.asarray(inputs[name])).max() == 0.0, (
            f"{name} is nonzero; this kernel folds only zero biases")
    wq_eff = wqkv * ln1_g[:, :, None]
    wf_eff = wfc_in * ln2_g[:, :, None]
    packed = dict(
        wqk=np.ascontiguousarray(wq_eff[:, :, :2 * D]).astype(NP_BF16),
        wv=np.ascontiguousarray(wq_eff[:, :, 2 * D:]).astype(NP_BF16),
        wpr=np.asarray(inputs["wproj"], np.float32).astype(NP_BF16),
        wfc=wf_eff.astype(NP_BF16),
        wf2=np.asarray(inputs["wfc2"], np.float32).astype(NP_BF16),
        wembT=np.ascontiguousarray(
            np.asarray(inputs["word_emb"], np.float32).T).astype(NP_BF16),
    )
    in_maps = []
    for b in range(B):
        bias = -10000.0 * (1.0 - mask[b])
        m = dict(packed)
        m["x0T"] = np.ascontiguousarray(x0[b].T).astype(np.float32)
        m["mb"] = np.ascontiguousarray(bias.reshape(TCH, P).T).astype(np.float32)
        in_maps.append(m)
    return in_maps


_NC_CACHE = {}


def get_nc():
    if "nc" not in _NC_CACHE:
        _NC_CACHE["nc"] = build(n_layers=12, with_head=True)
    return _NC_CACHE["nc"]


def kernel(**inputs) -> np.ndarray:
    nc = get_nc()
    in_maps = _prep_in_maps(inputs)
    res = bass_utils.run_bass_kernel_spmd(nc, in_maps, core_ids=list(range(NCORES)))
    return np.stack([res.results[b]["out"] for b in range(B)]).astype(np.float32)



# revision 26
# speedup vs baseline: 1.1321x; 1.1321x over previous
"""BERT-base (12-layer, B=8, T=512, D=768) forward + tied-embedding LM head
on 8 Trainium2 NeuronCores.

Sharding: data-parallel over batch -- core b computes batch element b
end-to-end (no collectives). Activations are feature-major [D, T] so GEMMs
consume weights in [d_in, d_out] layout; attention scores are key-major so
softmax denominators come from an appended ones-column on V.

v2 over the first working version:
  - softmax reciprocals batched per 6 heads on DVE (reciprocal_approx_fast)
    instead of per-head Ln/Exp on ACT -- kills ~350 ACT table loads.
  - a manual natural_log_exp table load per layer keeps ACT on one table set
    (gelu is the only other set, loaded once per layer under fc1).
  - V GEMM runs on the raw residual (stationary fp32 x) with a rank-1 mean
    correction matmul and a token-major rstd scale in the epilogue, so the
    PE has work while the LN scalar chain runs.
  - LN stats matmuls interleave with the previous GEMM's epilogue chunks.
  - QK weights are interleaved per head-pair so scores start after the
    first column group.
  - weight slabs host-packed contiguous; one DMA per column group.
  - logits emitted bf16.
"""

from contextlib import ExitStack

import numpy as np
import ml_dtypes

import concourse.bass as bass
import concourse.bacc as bacc
import concourse.mybir as mybir
import concourse.tile as tile
from concourse import bass_utils
from concourse._compat import get_trn_type
from concourse.hw_specs import get_activation_tables

NP_BF16 = ml_dtypes.bfloat16

FP32 = mybir.dt.float32
BF16 = mybir.dt.bfloat16
AF = mybir.ActivationFunctionType
OP = mybir.AluOpType

P = 128
T = 512
D = 768
H = 12
HD = 64
DF = 3072
V = 30522
VP = 30720       # vocab padded to 60*512
DK = D // P      # 6 contraction chunks over D
TCH = T // P     # 4 token chunks
FK = DF // P     # 24 contraction chunks over DF
SCALE = 0.125    # 1/sqrt(64)
EPS = 1e-5
NV = 512         # head vocab chunk


def bcast_ap(t, nparts, free):
    """partition-broadcast view of a [1, free] dram tile -> [nparts, free]"""
    return bass.AP(tensor=t.tensor, offset=t.offset, ap=[[0, nparts], [1, free]])


def build(n_layers=12, with_head=True, debug_taps=()):
    nc = bacc.Bacc(get_trn_type() or "TRN2", target_bir_lowering=False, debug=False)

    act_sets = list(get_activation_tables(nc.m.arch))
    NLE_SET = act_sets.index("natural_log_exp_and_others")

    def load_act_set(set_id):
        nc.scalar.add_instruction(
            mybir.InstLoadActFuncSet(
                name=nc.get_next_instruction_name(), act_func_set_id=set_id,
                ins=[], outs=[]))

    x0T = nc.dram_tensor("x0T", [D, T], FP32, kind="ExternalInput")
    mb = nc.dram_tensor("mb", [P, TCH], FP32, kind="ExternalInput")
    L = max(n_layers, 1)
    # host-packed contiguous weight slabs
    wqk = nc.dram_tensor("wqk", [L, DK, P, DK, 256], BF16, kind="ExternalInput")
    wv = nc.dram_tensor("wv", [L, P, DK, D], BF16, kind="ExternalInput")
    csv = nc.dram_tensor("csv", [L, 1, D], BF16, kind="ExternalInput")
    wpr = nc.dram_tensor("wpr", [L, P, DK, D], BF16, kind="ExternalInput")
    wfc = nc.dram_tensor("wfc", [L, 6, P, DK, 512], BF16, kind="ExternalInput")
    wf2 = nc.dram_tensor("wf2", [L, 6, P, FK, 128], BF16, kind="ExternalInput")
    if with_head:
        whd = nc.dram_tensor("whd", [VP // NV, P, DK, NV], BF16, kind="ExternalInput")
        out = nc.dram_tensor("out", [T, V], BF16, kind="ExternalOutput")
    else:
        out = nc.dram_tensor("out", [D, T], FP32, kind="ExternalOutput")

    tap_specs = {
        "h1": ([D, T], BF16), "qkT": ([2 * D, T], BF16),
        "v65": ([TCH * P, H * (HD + 1)], BF16), "p0": ([TCH * P, T], BF16),
        "yT": ([D, T], BF16), "xattn": ([D, T], FP32), "h2": ([D, T], BF16),
        "gT": ([DF, T], BF16), "xfinal": ([D, T], FP32),
        "den": ([H, T], BF16), "inv": ([H, T], BF16),
    }
    taps = {}
    for name in debug_taps:
        shape, dt = tap_specs[name]
        taps[name] = nc.dram_tensor(f"tap_{name}", shape, dt, kind="ExternalOutput")

    with tile.TileContext(nc) as tc, ExitStack() as ctx:
        consts = ctx.enter_context(tc.tile_pool(name="consts", bufs=1))
        resid = ctx.enter_context(tc.tile_pool(name="resid", bufs=1))
        acts = ctx.enter_context(tc.tile_pool(name="acts", bufs=2))
        wpool = ctx.enter_context(tc.tile_pool(name="wpool", bufs=1))
        small = ctx.enter_context(tc.tile_pool(name="small", bufs=1))
        ps_stat = ctx.enter_context(tc.tile_pool(name="ps_stat", bufs=2, space="PSUM"))
        ps_a = ctx.enter_context(tc.tile_pool(name="ps_a", bufs=2, space="PSUM"))
        ps_b = ctx.enter_context(tc.tile_pool(name="ps_b", bufs=2, space="PSUM"))
        ps_av = ctx.enter_context(tc.tile_pool(name="ps_av", bufs=2, space="PSUM"))
        dscratch = ctx.enter_context(tc.tile_pool(name="dscratch", bufs=2, space="DRAM"))

        czero = consts.tile([P, 1], FP32, tag="czero")
        nc.vector.memset(czero[:], 0.0)
        ceps = consts.tile([P, 1], FP32, tag="ceps")
        nc.vector.memset(ceps[:], EPS)
        nc.const_aps.aps[(FP32, 0.0)] = czero[:]
        nc.const_aps.aps[(FP32, EPS)] = ceps[:]

        cmean = consts.tile([P, 1], FP32, tag="cmean")     # stats lhsT: -1/D
        nc.vector.memset(cmean[:], -1.0 / D)
        ones_bf = consts.tile([P, 1], BF16, tag="ones_bf")
        nc.vector.memset(ones_bf[:], 1.0)
        ones_row = consts.tile([1, P], FP32, tag="ones_row")
        nc.vector.memset(ones_row[:], 1.0)
        ones_n1 = consts.tile([1, 1], FP32, tag="ones_n1")
        nc.vector.memset(ones_n1[:], 1.0)
        mb_sb = consts.tile([P, TCH], FP32, tag="mb_sb")
        nc.sync.dma_start(mb_sb[:], mb[:])

        # residual stream
        xT = []
        for j in range(DK):
            t = resid.tile([P, T], FP32, tag=f"x{j}")
            nc.sync.dma_start(t[:], x0T[j * P:(j + 1) * P, :])
            xT.append(t)

        def make_stats(tag):
            sum_ps = ps_stat.tile([1, T], FP32, tag="st", name=f"sum_{tag}")
            ssq_ps = ps_stat.tile([1, T], FP32, tag="st", name=f"ssq_{tag}")
            return sum_ps, ssq_ps

        xvb = [None] * DK

        def ln_stats_chunk(stp, j):
            """emit stats contributions for residual chunk j (x updated)."""
            sum_ps, ssq_ps = stp
            sq = acts.tile([P, T], BF16, tag="sq", bufs=2)
            nc.scalar.activation(sq[:], xT[j][:], AF.Square)
            nc.tensor.matmul(sum_ps[:], cmean[:], xT[j][:],
                             start=(j == 0), stop=(j == DK - 1))
            nc.tensor.matmul(ssq_ps[:], ones_bf[:], sq[:],
                             start=(j == 0), stop=(j == DK - 1))

        def xvb_chunk(j):
            xb = acts.tile([P, T], BF16, tag="xvb", bufs=6, name=f"xvb{j}")
            nc.vector.tensor_copy(xb[:], xT[j][:])
            xvb[j] = xb

        def ln_finish(stp, tag, with_v=False):
            """scalar chain: sum_ps holds -mean. Returns (rstd_b psum,
            nmrs_sb sbuf [P,T] bf16, and for V: nm_bf, rstdT, rstd)."""
            sum_ps, ssq_ps = stp
            msq = small.tile([1, T], FP32, tag="msq")
            nc.scalar.activation(msq[:], sum_ps[:], AF.Square)
            var = small.tile([1, T], FP32, tag="var")
            nc.vector.scalar_tensor_tensor(
                out=var[:], in0=ssq_ps[:], scalar=1.0 / D, in1=msq[:],
                op0=OP.mult, op1=OP.subtract)
            lnv = small.tile([1, T], FP32, tag="lnv")
            nc.scalar.activation(lnv[:], var[:], AF.Ln, bias=EPS)
            rstd = small.tile([1, T], FP32, tag="rstd")
            nc.scalar.activation(rstd[:], lnv[:], AF.Exp, scale=-0.5)
            ret = {}
            if with_v:
                # token-major rstd via K=1 transpose matmuls
                rstdT_ps = ps_av.tile([P, TCH], FP32, tag="av", name="rstdT_ps")
                for tch in range(TCH):
                    nc.tensor.matmul(
                        rstdT_ps[:, tch:tch + 1],
                        rstd[0:1, tch * P:(tch + 1) * P], ones_n1[:],
                        start=(tch == 0), stop=(tch == TCH - 1),
                        skip_group_check=True)
                rstdT = small.tile([P, TCH], FP32, tag="rstdT", bufs=2,
                                   name=f"rstdT_{tag}")
                nc.vector.tensor_copy(rstdT[:], rstdT_ps[:])
                nm_bf = small.tile([1, T], BF16, tag="nm", bufs=2,
                                   name=f"nm_{tag}")
                nc.vector.tensor_copy(nm_bf[:], sum_ps[:])
                ret["rstdT"] = rstdT
                ret["nm_bf"] = nm_bf
            nmrs_f = small.tile([1, T], FP32, tag="nmrs", bufs=2,
                                name=f"nmrs_{tag}")
            nc.vector.tensor_mul(nmrs_f[:], sum_ps[:], rstd[:])
            rstd_b = ps_stat.tile([P, T], FP32, tag="st", name=f"rstdb_{tag}")
            nc.tensor.matmul(rstd_b[:], ones_row[:], rstd[:], start=True, stop=True)
            nmrs_ps = ps_stat.tile([P, T], FP32, tag="st", name=f"nmrsb_{tag}")
            nc.tensor.matmul(nmrs_ps[:], ones_row[:], nmrs_f[:], start=True, stop=True)
            nmrs_sb = acts.tile([P, T], BF16, tag="nmrs_sb", bufs=2)
            nc.vector.tensor_copy(nmrs_sb[:], nmrs_ps[:])
            ret["rstd_b"] = rstd_b
            ret["nmrs_sb"] = nmrs_sb
            ret["rstd"] = rstd
            return ret

        def ln_apply(fin, tag):
            h_tiles = []
            for j in range(DK):
                h = acts.tile([P, T], BF16, tag="h", bufs=6, name=f"h_{tag}{j}")
                nc.vector.tensor_mul(h[:], xT[j][:], fin["rstd_b"][:])
                nc.vector.tensor_add(h[:], h[:], fin["nmrs_sb"][:])
                h_tiles.append(h)
            return h_tiles

        def dump_tiles(name, tiles, rows=P):
            if name in taps:
                for j, t in enumerate(tiles):
                    nc.sync.dma_start(taps[name][j * rows:(j + 1) * rows, :], t[:])

        def layer(l, stp1, last=False):
            # ---------- ln1 scalar chain (stats already accumulated) ----------
            fin1 = ln_finish(stp1, f"ln1_{l}", with_v=True)

            # ---------- V gemm on raw x (runs on PE during the chain) --------
            wv_sb = wpool.tile([P, DK, D], BF16, tag="wv", bufs=1)
            nc.sync.dma_start(wv_sb[:], wv[l])
            csv_sb = wpool.tile([1, D], BF16, tag="csv", bufs=1)
            nc.sync.dma_start(csv_sb[:], csv[l])
            v65 = []
            for tch in range(TCH):
                vt = acts.tile([P, H, HD + 1], BF16, tag="v65", bufs=4)
                nc.vector.memset(vt[:, :, HD:HD + 1], 1.0)
                for n0 in range(0, D, T):
                    nn = min(T, D - n0)
                    ps = (ps_a if n0 == 0 else ps_b).tile([P, T], FP32, tag="g")
                    for k in range(DK):
                        nc.tensor.matmul(
                            ps[:, :nn],
                            xvb[k][:, tch * P:(tch + 1) * P],
                            wv_sb[:, k, n0:n0 + nn],
                            start=(k == 0), stop=False)
                    # rank-1 mean correction: += (-mu)[t] * colsum(Wv)[c]
                    nc.tensor.matmul(
                        ps[:, :nn],
                        fin1["nm_bf"][0:1, tch * P:(tch + 1) * P],
                        csv_sb[0:1, n0:n0 + nn],
                        start=False, stop=True)
                    dst = vt[:, n0 // HD:(n0 + nn) // HD, 0:HD]
                    src = ps[:, :nn].rearrange("p (h d) -> p h d", d=HD)
                    nc.vector.tensor_scalar_mul(dst, src, fin1["rstdT"][:, tch:tch + 1])
                v65.append(vt)
            if "v65" in taps:
                for j, t in enumerate(v65):
                    nc.sync.dma_start(
                        taps["v65"][j * P:(j + 1) * P, :],
                        t[:].rearrange("p h d -> p (h d)"))

            # ---------- ln1 apply + QK gemm (pair-interleaved groups) --------
            h1 = ln_apply(fin1, f"ln1_{l}")
            dump_tiles("h1", h1)

            qkT = []
            for g in range(DK):
                s = wpool.tile([P, DK, 256], BF16, tag="wqk", bufs=2)
                nc.sync.dma_start(s[:], wqk[l, g])
                for mi in range(2):
                    ps = ps_a.tile([P, T], FP32, tag="g")
                    for k in range(DK):
                        nc.tensor.matmul(ps[:], s[:, k, mi * P:(mi + 1) * P],
                                         h1[k][:], start=(k == 0), stop=(k == DK - 1))
                    qt = acts.tile([P, T], BF16, tag="qkT", bufs=24, name=f"qkT{g}_{mi}")
                    nc.vector.tensor_copy(qt[:], ps[:])
                    qkT.append(qt)
            if "qkT" in taps:
                # qkT[2g]=q pair g, qkT[2g+1]=k pair g; dump q chunks then k
                for g in range(DK):
                    nc.sync.dma_start(taps["qkT"][g * P:(g + 1) * P, :], qkT[2 * g][:])
                    nc.sync.dma_start(
                        taps["qkT"][(DK + g) * P:(DK + g + 1) * P, :], qkT[2 * g + 1][:])

            # ---------- attention ----------
            yT = [acts.tile([P, T], BF16, tag="yT", bufs=6, name=f"yT{l}_{i}")
                  for i in range(DK)]
            ybufs = [None] * H
            den_t = [None, None]
            inv_d = [None, None]

            def emit_recip_batch(b):
                den_f = small.tile([6, T], FP32, tag="den_f")
                nc.vector.tensor_copy(den_f[:], den_t[b][:])
                inv_f = small.tile([6, T], FP32, tag="inv_f")
                nc.vector.reciprocal_approx_fast(inv_f[:], den_f[:])
                inv_bf = small.tile([6, T], BF16, tag="inv_bf")
                nc.vector.tensor_copy(inv_bf[:], inv_f[:])
                inv_d[b] = dscratch.tile([6, T], BF16, tag="inv_d",
                                         name=f"inv_d{l}_{b}")
                nc.sync.dma_start(inv_d[b][:], inv_bf[:])
                if "inv" in taps:
                    nc.sync.dma_start(taps["inv"][b * 6:(b + 1) * 6, :], inv_bf[:])

            def emit_norm(h):
                b, hh = h // 6, h % 6
                ht, r = h // 2, h % 2
                inv_h = acts.tile([HD, T], BF16, tag="inv_h", bufs=8)
                nc.sync.dma_start(inv_h[:], bcast_ap(inv_d[b][hh:hh + 1, :], HD, T))
                if r == 0:
                    nc.vector.tensor_mul(yT[ht][0:HD, :], ybufs[h][0:HD, :], inv_h[:])
                else:
                    ytmp = acts.tile([HD, T], BF16, tag="ytmp", bufs=2)
                    nc.vector.tensor_mul(ytmp[:], ybufs[h][0:HD, :], inv_h[:])
                    nc.sync.dma_start(yT[ht][HD:2 * HD, :], ytmp[:])

            for h in range(H):
                ht, r = h // 2, h % 2
                b, hh = h // 6, h % 6
                if hh == 0:
                    den_t[b] = acts.tile([6, T], BF16, tag=f"den{b}", bufs=2,
                                         name=f"den{l}_{b}")
                qt = qkT[2 * ht]
                kt = qkT[2 * ht + 1]
                rows = slice(r * HD, (r + 1) * HD)
                p_tiles = []
                for kc in range(TCH):
                    s_ps = ps_a.tile([P, T], FP32, tag="g")
                    nc.tensor.matmul(s_ps[:], kt[rows, kc * P:(kc + 1) * P],
                                     qt[rows, :], start=True, stop=True)
                    pt = acts.tile([P, T], BF16, tag="p", bufs=5)
                    nc.scalar.activation(pt[:], s_ps[:], AF.Exp,
                                         bias=mb_sb[:, kc:kc + 1], scale=SCALE)
                    p_tiles.append(pt)
                if h == 0 and "p0" in taps:
                    for kc in range(TCH):
                        nc.sync.dma_start(
                            taps["p0"][kc * P:(kc + 1) * P, :], p_tiles[kc][:])
                y_ps = ps_av.tile([HD + 1, T], FP32, tag="av")
                for kc in range(TCH):
                    nc.tensor.matmul(y_ps[:], v65[kc][:, h, :], p_tiles[kc][:],
                                     start=(kc == 0), stop=(kc == TCH - 1))
                yb = acts.tile([HD + 1, T], BF16, tag="ybuf", bufs=12)
                nc.vector.tensor_copy(yb[:], y_ps[:])
                ybufs[h] = yb
                nc.sync.dma_start(den_t[b][hh:hh + 1, :], yb[HD:HD + 1, :])
                if h == 5:
                    emit_recip_batch(0)
                if h == 8:
                    for hq in range(0, 6):
                        emit_norm(hq)
            emit_recip_batch(1)
            if "den" in taps:
                nc.sync.dma_start(taps["den"][0:6, :], den_t[0][:])
                nc.sync.dma_start(taps["den"][6:12, :], den_t[1][:])
            for hq in range(6, 12):
                emit_norm(hq)
            dump_tiles("yT", yT)

            # ---------- proj gemm + residual; ln2 stats interleaved ----------
            stp2 = make_stats(f"ln2_{l}")
            wpr_sb = wpool.tile([P, DK, D], BF16, tag="wpr", bufs=1)
            nc.sync.dma_start(wpr_sb[:], wpr[l])
            for m in range(DK):
                ps = ps_b.tile([P, T], FP32, tag="g")
                for k in range(DK):
                    nc.tensor.matmul(ps[:], wpr_sb[:, k, m * P:(m + 1) * P],
                                     yT[k][:], start=(k == 0), stop=(k == DK - 1))
                nc.vector.tensor_add(xT[m][:], xT[m][:], ps[:])
                ln_stats_chunk(stp2, m)
            dump_tiles("xattn", xT)

            # ---------- ln2 + fc1 + gelu ----------
            fin2 = ln_finish(stp2, f"ln2_{l}")
            h2 = ln_apply(fin2, f"ln2_{l}")
            dump_tiles("h2", h2)

            gT = []
            for g in range(6):
                s = wpool.tile([P, DK, 512], BF16, tag="wfc", bufs=2)
                nc.sync.dma_start(s[:], wfc[l, g])
                for mi in range(4):
                    ps = ps_a.tile([P, T], FP32, tag="g")
                    for k in range(DK):
                        nc.tensor.matmul(ps[:], s[:, k, mi * P:(mi + 1) * P],
                                         h2[k][:], start=(k == 0), stop=(k == DK - 1))
                    gt = acts.tile([P, T], BF16, tag="qkT", bufs=24,
                                   name=f"gT{l}_{g}_{mi}")
                    nc.scalar.activation(gt[:], ps[:], AF.Gelu_apprx_tanh)
                    gT.append(gt)
            dump_tiles("gT", gT)

            # switch ACT back to the ln/exp set for the next layer's chain
            # (hidden under the fc2 matmuls)
            load_act_set(NLE_SET)

            # ---------- fc2 gemm + residual; next ln1 stats interleaved ------
            stp_next = None if last else make_stats(f"ln1_{l + 1}")
            for g in range(6):
                s = wpool.tile([P, FK, 128], BF16, tag="wf2", bufs=2)
                nc.sync.dma_start(s[:], wf2[l, g])
                m = g
                ps = ps_b.tile([P, T], FP32, tag="g")
                for k in range(FK):
                    nc.tensor.matmul(ps[:], s[:, k, :],
                                     gT[k][:], start=(k == 0), stop=(k == FK - 1))
                nc.vector.tensor_add(xT[m][:], xT[m][:], ps[:])
                if not last:
                    ln_stats_chunk(stp_next, m)
                if not last or with_head:
                    xvb_chunk(m)
            return stp_next

        # initial table set + ln1 stats of layer 0
        load_act_set(NLE_SET)
        stp = make_stats("ln1_0")
        for j in range(DK):
            ln_stats_chunk(stp, j)
            xvb_chunk(j)

        for l in range(n_layers):
            stp = layer(l, stp, last=(l == n_layers - 1))
        dump_tiles("xfinal", xT)

        if not with_head:
            for j in range(DK):
                nc.sync.dma_start(out[j * P:(j + 1) * P, :], xT[j][:])
        else:
            # ---- LM head: logits[t, v] = x @ wembT, bf16 out ----
            xbf = xvb
            for vs in range(VP // NV):
                nn = min(NV, V - vs * NV)
                w_sb = wpool.tile([P, DK, NV], BF16, tag="whead", bufs=2)
                nc.sync.dma_start(w_sb[:], whd[vs])
                for tch in range(TCH):
                    ps = (ps_a if tch % 2 == 0 else ps_b).tile([P, NV], FP32, tag="g")
                    for k in range(DK):
                        nc.tensor.matmul(
                            ps[:], xbf[k][:, tch * P:(tch + 1) * P],
                            w_sb[:, k, :], start=(k == 0), stop=(k == DK - 1))
                    o = acts.tile([P, NV], BF16, tag="o_head", bufs=3)
                    nc.vector.tensor_copy(o[:], ps[:])
                    nc.sync.dma_start(out[tch * P:(tch + 1) * P,
                                          vs * NV:vs * NV + nn], o[:, :nn])

    nc.compile()
    return nc


# ---------------------------------------------------------------------------
# host side
# ---------------------------------------------------------------------------

B = 8
NCORES = 8


def _np_layer_norm(x, g, b, eps=1e-5):
    m = x.mean(-1, keepdims=True)
    v = x.var(-1, keepdims=True)
    return (x - m) / np.sqrt(v + eps) * g + b


def _pack_km(w, cg):
    """[L, Din, M] -> [L, M//cg, 128, Din//128, cg] contiguous slabs"""
    Lx, Din, M = w.shape
    nk = Din // P
    g = M // cg
    # [L, nk, ki, gi, c] -> [L, gi, ki, nk, c]
    r = w.reshape(Lx, nk, P, g, cg).transpose(0, 3, 2, 1, 4)
    return np.ascontiguousarray(r)


def _prep_in_maps(inputs):
    ids = np.asarray(inputs["input_ids"]).astype(np.int64)
    tt = np.asarray(inputs["token_type_ids"]).astype(np.int64)
    x0 = (np.asarray(inputs["word_emb"], np.float32)[ids]
          + np.asarray(inputs["pos_emb"], np.float32)[None, :ids.shape[1], :]
          + np.asarray(inputs["type_emb"], np.float32)[tt])
    x0 = _np_layer_norm(x0, np.asarray(inputs["emb_ln_g"], np.float32),
                        np.asarray(inputs["emb_ln_b"], np.float32))
    mask = np.asarray(inputs["attention_mask"], np.float32)

    wqkv = np.asarray(inputs["wqkv"], np.float32)
    wfc_in = np.asarray(inputs["wfc"], np.float32)
    ln1_g = np.asarray(inputs["ln1_g"], np.float32)
    ln2_g = np.asarray(inputs["ln2_g"], np.float32)
    for name in ("bqkv", "bproj", "bfc", "bfc2", "ln1_b", "ln2_b"):
        assert np.abs(np.asarray(inputs[name])).max() == 0.0, (
            f"{name} is nonzero; this kernel folds only zero biases")
    wq_eff = wqkv * ln1_g[:, :, None]
    wf_eff = wfc_in * ln2_g[:, :, None]
    Lx = wqkv.shape[0]

    # interleave q/k per head pair: group g holds [q_pair_g | k_pair_g]
    q = wq_eff[:, :, 0 * D:1 * D].reshape(Lx, D, DK, P)
    k = wq_eff[:, :, 1 * D:2 * D].reshape(Lx, D, DK, P)
    qk_il = np.concatenate([q, k], axis=3)          # [L, D, 6, 256]
    qk_il = qk_il.reshape(Lx, D, 2 * D)
    wv_eff = np.ascontiguousarray(wq_eff[:, :, 2 * D:])

    wemb = np.asarray(inputs["word_emb"], np.float32)     # [V, D]
    wembT = np.zeros((D, VP), np.float32)
    wembT[:, :V] = wemb.T
    whd = _pack_km(wembT[None], NV)[0]                    # [60, 128, 6, 512]

    packed = dict(
        wqk=_pack_km(qk_il, 256).astype(NP_BF16),
        wv=np.ascontiguousarray(
            wv_eff.reshape(Lx, DK, P, D).transpose(0, 2, 1, 3)).astype(NP_BF16),
        csv=np.ascontiguousarray(wv_eff.sum(axis=1)[:, None, :]).astype(NP_BF16),
        wpr=np.ascontiguousarray(
            np.asarray(inputs["wproj"], np.float32)
            .reshape(Lx, DK, P, D).transpose(0, 2, 1, 3)).astype(NP_BF16),
        wfc=_pack_km(wf_eff, 512).astype(NP_BF16),
        wf2=_pack_km(np.asarray(inputs["wfc2"], np.float32), 128).astype(NP_BF16),
        whd=whd.astype(NP_BF16),
    )
    in_maps = []
    for b in range(B):
        bias = -10000.0 * (1.0 - mask[b])
        m = dict(packed)
        m["x0T"] = np.ascontiguousarray(x0[b].T).astype(np.float32)
        m["mb"] = np.ascontiguousarray(bias.reshape(TCH, P).T).astype(np.float32)
        in_maps.append(m)
    return in_maps


_NC_CACHE = {}


def get_nc():
    if "nc" not in _NC_CACHE:
        _NC_CACHE["nc"] = build(n_layers=12, with_head=True)
    return _NC_CACHE["nc"]


def kernel(**inputs) -> np.ndarray:
    nc = get_nc()
    in_maps = _prep_in_maps(inputs)
    res = bass_utils.run_bass_kernel_spmd(nc, in_maps, core_ids=list(range(NCORES)))
    return np.stack([res.results[b]["out"] for b in range(B)]).astype(np.float32)


# revision 33
# speedup vs baseline: 2.3780x; 2.1006x over previous
"""BERT-base (12-layer, B=8, T=512, D=768) forward + tied-embedding LM head
on 8 Trainium2 NeuronCores.

Sharding: data-parallel over batch -- core b computes batch element b
end-to-end (no collectives). Activations are feature-major [D, T] so GEMMs
consume weights in [d_in, d_out] layout; attention scores are key-major so
softmax denominators come from an appended ones-column on V.

v2 over the first working version:
  - softmax reciprocals batched per 6 heads on DVE (reciprocal_approx_fast)
    instead of per-head Ln/Exp on ACT -- kills ~350 ACT table loads.
  - a manual natural_log_exp table load per layer keeps ACT on one table set
    (gelu is the only other set, loaded once per layer under fc1).
  - V GEMM runs on the raw residual (stationary fp32 x) with a rank-1 mean
    correction matmul and a token-major rstd scale in the epilogue, so the
    PE has work while the LN scalar chain runs.
  - LN stats matmuls interleave with the previous GEMM's epilogue chunks.
  - QK weights are interleaved per head-pair so scores start after the
    first column group.
  - weight slabs host-packed contiguous; one DMA per column group.
  - logits emitted bf16.
"""

from contextlib import ExitStack

import numpy as np
import ml_dtypes

import concourse.bass as bass
import concourse.bacc as bacc
import concourse.mybir as mybir
import concourse.tile as tile
from concourse import bass_utils
from concourse._compat import get_trn_type
from concourse.hw_specs import get_activation_tables

NP_BF16 = ml_dtypes.bfloat16

FP32 = mybir.dt.float32
BF16 = mybir.dt.bfloat16
AF = mybir.ActivationFunctionType
OP = mybir.AluOpType

P = 128
T = 512
D = 768
H = 12
HD = 64
DF = 3072
V = 30522
VP = 30720       # vocab padded to 60*512
DK = D // P      # 6 contraction chunks over D
TCH = T // P     # 4 token chunks
FK = DF // P     # 24 contraction chunks over DF
SCALE = 0.125    # 1/sqrt(64)
EPS = 1e-5
NV = 512         # head vocab chunk


def bcast_ap(t, nparts, free):
    """partition-broadcast view of a [1, free] dram tile -> [nparts, free]"""
    return bass.AP(tensor=t.tensor, offset=t.offset, ap=[[0, nparts], [1, free]])


def build(n_layers=12, with_head=True, debug_taps=()):
    nc = bacc.Bacc(get_trn_type() or "TRN2", target_bir_lowering=False, debug=False)

    act_sets = list(get_activation_tables(nc.m.arch))
    NLE_SET = act_sets.index("natural_log_exp_and_others")

    def load_act_set(set_id):
        nc.scalar.add_instruction(
            mybir.InstLoadActFuncSet(
                name=nc.get_next_instruction_name(), act_func_set_id=set_id,
                ins=[], outs=[]))

    x0T = nc.dram_tensor("x0T", [D, T], FP32, kind="ExternalInput")
    mb = nc.dram_tensor("mb", [P, TCH], FP32, kind="ExternalInput")
    L = max(n_layers, 1)
    # host-packed contiguous weight slabs
    wqk = nc.dram_tensor("wqk", [L, DK, P, DK, 256], BF16, kind="ExternalInput")
    wv = nc.dram_tensor("wv", [L, P, DK, D], BF16, kind="ExternalInput")
    csv = nc.dram_tensor("csv", [L, 1, D], BF16, kind="ExternalInput")
    wpr = nc.dram_tensor("wpr", [L, P, DK, D], BF16, kind="ExternalInput")
    wfc = nc.dram_tensor("wfc", [L, 6, P, DK, 512], BF16, kind="ExternalInput")
    wf2 = nc.dram_tensor("wf2", [L, 6, P, FK, 128], BF16, kind="ExternalInput")
    if with_head:
        whd = nc.dram_tensor("whd", [VP // NV, P, DK, NV], BF16, kind="ExternalInput")
        out = nc.dram_tensor("out", [T, V], BF16, kind="ExternalOutput")
    else:
        out = nc.dram_tensor("out", [D, T], FP32, kind="ExternalOutput")

    tap_specs = {
        "h1": ([D, T], BF16), "qkT": ([2 * D, T], BF16),
        "v65": ([TCH * P, H * (HD + 1)], BF16), "p0": ([TCH * P, T], BF16),
        "yT": ([D, T], BF16), "xattn": ([D, T], FP32), "h2": ([D, T], BF16),
        "gT": ([DF, T], BF16), "xfinal": ([D, T], FP32),
        "den": ([H, T], BF16), "inv": ([H, T], BF16),
    }
    taps = {}
    for name in debug_taps:
        shape, dt = tap_specs[name]
        taps[name] = nc.dram_tensor(f"tap_{name}", shape, dt, kind="ExternalOutput")

    with tile.TileContext(nc) as tc, ExitStack() as ctx:
        consts = ctx.enter_context(tc.tile_pool(name="consts", bufs=1))
        resid = ctx.enter_context(tc.tile_pool(name="resid", bufs=1))
        acts = ctx.enter_context(tc.tile_pool(name="acts", bufs=2))
        wpool = ctx.enter_context(tc.tile_pool(name="wpool", bufs=1))
        small = ctx.enter_context(tc.tile_pool(name="small", bufs=1))
        # ps_a slots are 2 banks ([P,1024] score/gelu pairs also live here);
        # ps_sa is shared between LN stats/broadcasts and attention AV tiles
        # (disjoint lifetimes).
        ps_a = ctx.enter_context(tc.tile_pool(name="ps_a", bufs=2, space="PSUM"))
        ps_b = ctx.enter_context(tc.tile_pool(name="ps_b", bufs=2, space="PSUM"))
        ps_sa = ctx.enter_context(tc.tile_pool(name="ps_sa", bufs=2, space="PSUM"))
        ps_stat = ps_sa
        ps_av = ps_sa
        dscratch = ctx.enter_context(tc.tile_pool(name="dscratch", bufs=2, space="DRAM"))

        czero = consts.tile([P, 1], FP32, tag="czero")
        nc.vector.memset(czero[:], 0.0)
        ceps = consts.tile([P, 1], FP32, tag="ceps")
        nc.vector.memset(ceps[:], EPS)
        nc.const_aps.aps[(FP32, 0.0)] = czero[:]
        nc.const_aps.aps[(FP32, EPS)] = ceps[:]

        cmean = consts.tile([P, 1], FP32, tag="cmean")     # stats lhsT: -1/D
        nc.vector.memset(cmean[:], -1.0 / D)
        ones_bf = consts.tile([P, 1], BF16, tag="ones_bf")
        nc.vector.memset(ones_bf[:], 1.0)
        ones_row = consts.tile([1, P], FP32, tag="ones_row")
        nc.vector.memset(ones_row[:], 1.0)
        ones_n1 = consts.tile([1, 1], FP32, tag="ones_n1")
        nc.vector.memset(ones_n1[:], 1.0)
        mb_sb = consts.tile([P, TCH], FP32, tag="mb_sb")
        nc.sync.dma_start(mb_sb[:], mb[:])

        # residual stream
        xT = []
        for j in range(DK):
            t = resid.tile([P, T], FP32, tag=f"x{j}")
            nc.sync.dma_start(t[:], x0T[j * P:(j + 1) * P, :])
            xT.append(t)

        def make_stats(tag):
            sum_ps = ps_stat.tile([1, T], FP32, tag="sa", name=f"sum_{tag}")
            ssq_ps = ps_stat.tile([1, T], FP32, tag="sa", name=f"ssq_{tag}")
            return sum_ps, ssq_ps

        xvb = [None] * DK

        def ln_stats_chunk(stp, j):
            """emit stats contributions for residual chunk j (x updated)."""
            sum_ps, ssq_ps = stp
            sq = acts.tile([P, T], BF16, tag="sq", bufs=2)
            nc.scalar.activation(sq[:], xT[j][:], AF.Square)
            nc.tensor.matmul(sum_ps[:], cmean[:], xT[j][:],
                             start=(j == 0), stop=(j == DK - 1))
            nc.tensor.matmul(ssq_ps[:], ones_bf[:], sq[:],
                             start=(j == 0), stop=(j == DK - 1))

        def xvb_chunk(j):
            xb = acts.tile([P, T], BF16, tag="xvb", bufs=6, name=f"xvb{j}")
            nc.vector.tensor_copy(xb[:], xT[j][:])
            xvb[j] = xb

        def ln_finish(stp, tag, with_v=False):
            """scalar chain: sum_ps holds -mean. Returns (rstd_b psum,
            nmrs_sb sbuf [P,T] bf16, and for V: nm_bf, rstdT, rstd)."""
            sum_ps, ssq_ps = stp
            msq = small.tile([1, T], FP32, tag="msq")
            nc.scalar.activation(msq[:], sum_ps[:], AF.Square)
            var = small.tile([1, T], FP32, tag="var")
            nc.vector.scalar_tensor_tensor(
                out=var[:], in0=ssq_ps[:], scalar=1.0 / D, in1=msq[:],
                op0=OP.mult, op1=OP.subtract)
            lnv = small.tile([1, T], FP32, tag="lnv")
            nc.scalar.activation(lnv[:], var[:], AF.Ln, bias=EPS)
            rstd = small.tile([1, T], FP32, tag="rstd")
            nc.scalar.activation(rstd[:], lnv[:], AF.Exp, scale=-0.5)
            ret = {}
            if with_v:
                # token-major rstd via K=1 transpose matmuls
                rstdT_ps = ps_av.tile([P, TCH], FP32, tag="sa", name="rstdT_ps")
                for tch in range(TCH):
                    nc.tensor.matmul(
                        rstdT_ps[:, tch:tch + 1],
                        rstd[0:1, tch * P:(tch + 1) * P], ones_n1[:],
                        start=(tch == 0), stop=(tch == TCH - 1),
                        skip_group_check=True)
                rstdT = small.tile([P, TCH], FP32, tag="rstdT", bufs=2,
                                   name=f"rstdT_{tag}")
                nc.vector.tensor_copy(rstdT[:], rstdT_ps[:])
                nm_bf = small.tile([1, T], BF16, tag="nm", bufs=2,
                                   name=f"nm_{tag}")
                nc.vector.tensor_copy(nm_bf[:], sum_ps[:])
                ret["rstdT"] = rstdT
                ret["nm_bf"] = nm_bf
            nmrs_f = small.tile([1, T], FP32, tag="nmrs", bufs=2,
                                name=f"nmrs_{tag}")
            nc.vector.tensor_mul(nmrs_f[:], sum_ps[:], rstd[:])
            rstd_b = ps_stat.tile([P, T], FP32, tag="sa", name=f"rstdb_{tag}")
            nc.tensor.matmul(rstd_b[:], ones_row[:], rstd[:], start=True, stop=True)
            nmrs_ps = ps_stat.tile([P, T], FP32, tag="sa", name=f"nmrsb_{tag}")
            nc.tensor.matmul(nmrs_ps[:], ones_row[:], nmrs_f[:], start=True, stop=True)
            nmrs_sb = acts.tile([P, T], BF16, tag="nmrs_sb", bufs=2)
            nc.vector.tensor_copy(nmrs_sb[:], nmrs_ps[:])
            ret["rstd_b"] = rstd_b
            ret["nmrs_sb"] = nmrs_sb
            ret["rstd"] = rstd
            return ret

        def ln_apply(fin, tag):
            h_tiles = []
            for j in range(DK):
                h = acts.tile([P, T], BF16, tag="h", bufs=6, name=f"h_{tag}{j}")
                nc.vector.tensor_mul(h[:], xT[j][:], fin["rstd_b"][:])
                nc.vector.tensor_add(h[:], h[:], fin["nmrs_sb"][:])
                h_tiles.append(h)
            return h_tiles

        def dump_tiles(name, tiles, rows=P):
            if name in taps:
                for j, t in enumerate(tiles):
                    nc.sync.dma_start(taps[name][j * rows:(j + 1) * rows, :], t[:])

        def layer(l, stp1, last=False):
            # ---------- ln1 scalar chain (stats already accumulated) ----------
            fin1 = ln_finish(stp1, f"ln1_{l}", with_v=True)

            # ---------- V gemm on raw x (runs on PE during the chain) --------
            wv_sb = wpool.tile([P, DK, D], BF16, tag="wv", bufs=1)
            nc.sync.dma_start(wv_sb[:], wv[l])
            csv_sb = wpool.tile([1, D], BF16, tag="csv", bufs=1)
            nc.sync.dma_start(csv_sb[:], csv[l])
            v65 = []
            for tch in range(TCH):
                vt = acts.tile([P, H, HD + 1], BF16, tag="v65", bufs=4)
                nc.vector.memset(vt[:, :, HD:HD + 1], 1.0)
                for n0 in range(0, D, T):
                    nn = min(T, D - n0)
                    ps = (ps_a if n0 == 0 else ps_b).tile([P, T], FP32, tag="g")
                    for k in range(DK):
                        nc.tensor.matmul(
                            ps[:, :nn],
                            xvb[k][:, tch * P:(tch + 1) * P],
                            wv_sb[:, k, n0:n0 + nn],
                            start=(k == 0), stop=False)
                    # rank-1 mean correction: += (-mu)[t] * colsum(Wv)[c]
                    nc.tensor.matmul(
                        ps[:, :nn],
                        fin1["nm_bf"][0:1, tch * P:(tch + 1) * P],
                        csv_sb[0:1, n0:n0 + nn],
                        start=False, stop=True)
                    dst = vt[:, n0 // HD:(n0 + nn) // HD, 0:HD]
                    src = ps[:, :nn].rearrange("p (h d) -> p h d", d=HD)
                    nc.vector.tensor_scalar_mul(dst, src, fin1["rstdT"][:, tch:tch + 1])
                v65.append(vt)
            if "v65" in taps:
                for j, t in enumerate(v65):
                    nc.sync.dma_start(
                        taps["v65"][j * P:(j + 1) * P, :],
                        t[:].rearrange("p h d -> p (h d)"))

            # ---------- ln1 apply + QK gemm (pair-interleaved groups) --------
            h1 = ln_apply(fin1, f"ln1_{l}")
            dump_tiles("h1", h1)

            qkT = []
            for g in range(DK):
                s = wpool.tile([P, DK, 256], BF16, tag="wqk", bufs=2)
                nc.sync.dma_start(s[:], wqk[l, g])
                for mi in range(2):
                    ps = ps_a.tile([P, T], FP32, tag="g")
                    for k in range(DK):
                        nc.tensor.matmul(ps[:], s[:, k, mi * P:(mi + 1) * P],
                                         h1[k][:], start=(k == 0), stop=(k == DK - 1))
                    qt = acts.tile([P, T], BF16, tag="qkT", bufs=12, name=f"qkT{g}_{mi}")
                    nc.vector.tensor_copy(qt[:], ps[:])
                    qkT.append(qt)
            if "qkT" in taps:
                # qkT[2g]=q pair g, qkT[2g+1]=k pair g; dump q chunks then k
                for g in range(DK):
                    nc.sync.dma_start(taps["qkT"][g * P:(g + 1) * P, :], qkT[2 * g][:])
                    nc.sync.dma_start(
                        taps["qkT"][(DK + g) * P:(DK + g + 1) * P, :], qkT[2 * g + 1][:])

            # ---------- attention (head pairs; denom batches of 8+4) --------
            # prefetch proj weights so proj can start the moment yT is ready
            wpr_sb = wpool.tile([P, DK, D], BF16, tag="wpr", bufs=1)
            nc.sync.dma_start(wpr_sb[:], wpr[l])

            yT = [acts.tile([P, T], BF16, tag="yT", bufs=6, name=f"yT{l}_{i}")
                  for i in range(DK)]
            ybufs = [None] * H
            den_t = [None, None]
            inv_d = [None, None]
            NB0 = 8   # heads in denominator batch 0

            def emit_recip_batch(b):
                nb = NB0 if b == 0 else H - NB0
                den_f = small.tile([NB0, T], FP32, tag="den_f", name=f"den_f{l}{b}")
                nc.vector.tensor_copy(den_f[:nb, :], den_t[b][:nb, :])
                inv_f = small.tile([NB0, T], FP32, tag="inv_f", name=f"inv_f{l}{b}")
                nc.vector.reciprocal_approx_fast(inv_f[:nb, :], den_f[:nb, :])
                inv_bf = small.tile([NB0, T], BF16, tag="inv_bf", name=f"inv_b{l}{b}")
                nc.vector.tensor_copy(inv_bf[:nb, :], inv_f[:nb, :])
                inv_d[b] = dscratch.tile([NB0, T], BF16, tag="inv_d",
                                         name=f"inv_d{l}_{b}")
                nc.sync.dma_start(inv_d[b][:nb, :], inv_bf[:nb, :])
                if "inv" in taps:
                    nc.sync.dma_start(taps["inv"][b * NB0:b * NB0 + nb, :],
                                      inv_bf[:nb, :])

            def emit_norm(h):
                b, hh = (0, h) if h < NB0 else (1, h - NB0)
                ht, r = h // 2, h % 2
                inv_h = acts.tile([HD, T], BF16, tag="inv_h", bufs=6)
                nc.sync.dma_start(inv_h[:], bcast_ap(inv_d[b][hh:hh + 1, :], HD, T))
                if r == 0:
                    nc.vector.tensor_mul(yT[ht][0:HD, :], ybufs[h][0:HD, :], inv_h[:])
                else:
                    ytmp = acts.tile([HD, T], BF16, tag="ytmp", bufs=2)
                    nc.vector.tensor_mul(ytmp[:], ybufs[h][0:HD, :], inv_h[:])
                    nc.sync.dma_start(yT[ht][HD:2 * HD, :], ytmp[:])

            for ht in range(H // 2):
                hA, hB = 2 * ht, 2 * ht + 1
                for h in (hA, hB):
                    b, hh = (0, h) if h < NB0 else (1, h - NB0)
                    if hh == 0:
                        den_t[b] = acts.tile([NB0, T], BF16, tag=f"den{b}", bufs=2,
                                             name=f"den{l}_{b}")
                qt = qkT[2 * ht]
                kt = qkT[2 * ht + 1]
                p_tiles = []
                for kc in range(TCH):
                    s_ps = ps_a.tile([P, 2 * T], FP32, tag="g")
                    for r in (0, 1):
                        nc.tensor.matmul(
                            s_ps[:, r * T:(r + 1) * T],
                            kt[r * HD:(r + 1) * HD, kc * P:(kc + 1) * P],
                            qt[r * HD:(r + 1) * HD, :], start=True, stop=True)
                    pt = acts.tile([P, 2 * T], BF16, tag="p", bufs=4)
                    nc.scalar.activation(pt[:], s_ps[:], AF.Exp,
                                         bias=mb_sb[:, kc:kc + 1], scale=SCALE)
                    p_tiles.append(pt)
                if ht == 0 and "p0" in taps:
                    for kc in range(TCH):
                        nc.sync.dma_start(
                            taps["p0"][kc * P:(kc + 1) * P, :], p_tiles[kc][:, 0:T])
                for r, h in ((0, hA), (1, hB)):
                    b, hh = (0, h) if h < NB0 else (1, h - NB0)
                    y_ps = ps_av.tile([HD + 1, T], FP32, tag="sa")
                    for kc in range(TCH):
                        nc.tensor.matmul(y_ps[:], v65[kc][:, h, :],
                                         p_tiles[kc][:, r * T:(r + 1) * T],
                                         start=(kc == 0), stop=(kc == TCH - 1))
                    yb = acts.tile([HD + 1, T], BF16, tag="ybuf", bufs=10)
                    nc.vector.tensor_copy(yb[:], y_ps[:])
                    ybufs[h] = yb
                    nc.sync.dma_start(den_t[b][hh:hh + 1, :], yb[HD:HD + 1, :])
                if ht == 3:
                    emit_recip_batch(0)
                if ht == 4:
                    for hq in range(0, NB0):
                        emit_norm(hq)
            emit_recip_batch(1)
            if "den" in taps:
                nc.sync.dma_start(taps["den"][0:NB0, :], den_t[0][:])
                nc.sync.dma_start(taps["den"][NB0:H, :], den_t[1][:H - NB0, :])
            for hq in range(NB0, H):
                emit_norm(hq)
            dump_tiles("yT", yT)

            # ---------- proj gemm + residual; ln2 stats interleaved ----------
            # 4 chains emitted k-major so the PE has 16 ready matmuls (k 0..3
            # come from denominator batch 0) while batch 1 normalizes.
            stp2 = make_stats(f"ln2_{l}")
            pchain = []
            for m in range(4):
                pchain.append((ps_b if m % 2 == 0 else ps_a)
                              .tile([P, T], FP32, tag="g", name=f"prj{l}_{m}"))
            for k in range(DK):
                for m in range(4):
                    nc.tensor.matmul(pchain[m][:], wpr_sb[:, k, m * P:(m + 1) * P],
                                     yT[k][:], start=(k == 0), stop=(k == DK - 1))
            for m in range(4):
                nc.vector.tensor_add(xT[m][:], xT[m][:], pchain[m][:])
                ln_stats_chunk(stp2, m)
            for m in range(4, DK):
                ps = (ps_b if m % 2 == 0 else ps_a).tile([P, T], FP32, tag="g")
                for k in range(DK):
                    nc.tensor.matmul(ps[:], wpr_sb[:, k, m * P:(m + 1) * P],
                                     yT[k][:], start=(k == 0), stop=(k == DK - 1))
                nc.vector.tensor_add(xT[m][:], xT[m][:], ps[:])
                ln_stats_chunk(stp2, m)
            dump_tiles("xattn", xT)

            # ---------- ln2 + fc1 + gelu ----------
            fin2 = ln_finish(stp2, f"ln2_{l}")
            h2 = ln_apply(fin2, f"ln2_{l}")
            dump_tiles("h2", h2)

            gT = []   # 12 tiles [P, 2T]; df-chunk k lives in gT[k//2][:, (k%2)*T:]
            for g in range(6):
                s = wpool.tile([P, DK, 512], BF16, tag="wfc", bufs=2)
                nc.sync.dma_start(s[:], wfc[l, g])
                for mp in range(2):
                    ps = ps_a.tile([P, 2 * T], FP32, tag="g")
                    for mi2 in range(2):
                        mi = 2 * mp + mi2
                        for k in range(DK):
                            nc.tensor.matmul(
                                ps[:, mi2 * T:(mi2 + 1) * T],
                                s[:, k, mi * P:(mi + 1) * P],
                                h2[k][:], start=(k == 0), stop=(k == DK - 1))
                    gt = acts.tile([P, 2 * T], BF16, tag="gbig", bufs=12,
                                   name=f"gT{l}_{g}_{mp}")
                    nc.scalar.activation(gt[:], ps[:], AF.Gelu_apprx_tanh)
                    gT.append(gt)
            if "gT" in taps:
                for j, t in enumerate(gT):
                    nc.sync.dma_start(taps["gT"][j * 2 * P:(j * 2 + 1) * P, :],
                                      t[:, 0:T])
                    nc.sync.dma_start(taps["gT"][(j * 2 + 1) * P:(j * 2 + 2) * P, :],
                                      t[:, T:2 * T])

            # switch ACT back to the ln/exp set for the next layer's chain
            # (hidden under the fc2 matmuls)
            load_act_set(NLE_SET)

            # ---------- fc2 gemm + residual; next ln1 stats interleaved ------
            stp_next = None if last else make_stats(f"ln1_{l + 1}")
            for g in range(6):
                s = wpool.tile([P, FK, 128], BF16, tag="wf2", bufs=2)
                nc.sync.dma_start(s[:], wf2[l, g])
                m = g
                ps = ps_b.tile([P, T], FP32, tag="g")
                for k in range(FK):
                    nc.tensor.matmul(ps[:], s[:, k, :],
                                     gT[k // 2][:, (k % 2) * T:(k % 2 + 1) * T],
                                     start=(k == 0), stop=(k == FK - 1))
                nc.vector.tensor_add(xT[m][:], xT[m][:], ps[:])
                if not last:
                    ln_stats_chunk(stp_next, m)
                if not last or with_head:
                    xvb_chunk(m)
            return stp_next

        # initial table set + ln1 stats of layer 0
        load_act_set(NLE_SET)
        stp = make_stats("ln1_0")
        for j in range(DK):
            ln_stats_chunk(stp, j)
            xvb_chunk(j)

        for l in range(n_layers):
            stp = layer(l, stp, last=(l == n_layers - 1))
        dump_tiles("xfinal", xT)

        if not with_head:
            for j in range(DK):
                nc.sync.dma_start(out[j * P:(j + 1) * P, :], xT[j][:])
        else:
            # ---- LM head: logits[t, v] = x @ wembT, bf16 out ----
            xbf = xvb
            for vs in range(VP // NV):
                nn = min(NV, V - vs * NV)
                w_sb = wpool.tile([P, DK, NV], BF16, tag="whead", bufs=2)
                nc.sync.dma_start(w_sb[:], whd[vs])
                for tch in range(TCH):
                    ps = (ps_a if tch % 2 == 0 else ps_b).tile([P, NV], FP32, tag="g")
                    for k in range(DK):
                        nc.tensor.matmul(
                            ps[:], xbf[k][:, tch * P:(tch + 1) * P],
                            w_sb[:, k, :], start=(k == 0), stop=(k == DK - 1))
                    o = acts.tile([P, NV], BF16, tag="o_head", bufs=3)
                    nc.vector.tensor_copy(o[:], ps[:])
                    nc.sync.dma_start(out[tch * P:(tch + 1) * P,
                                          vs * NV:vs * NV + nn], o[:, :nn])

    nc.compile()
    return nc


# ---------------------------------------------------------------------------
# host side
# ---------------------------------------------------------------------------

B = 8
NCORES = 8


def _np_layer_norm(x, g, b, eps=1e-5):
    m = x.mean(-1, keepdims=True)
    v = x.var(-1, keepdims=True)
    return (x - m) / np.sqrt(v + eps) * g + b


def _pack_km(w, cg):
    """[L, Din, M] -> [L, M//cg, 128, Din//128, cg] contiguous slabs"""
    Lx, Din, M = w.shape
    nk = Din // P
    g = M // cg
    # [L, nk, ki, gi, c] -> [L, gi, ki, nk, c]
    r = w.reshape(Lx, nk, P, g, cg).transpose(0, 3, 2, 1, 4)
    return np.ascontiguousarray(r)


def _prep_in_maps(inputs):
    ids = np.asarray(inputs["input_ids"]).astype(np.int64)
    tt = np.asarray(inputs["token_type_ids"]).astype(np.int64)
    x0 = (np.asarray(inputs["word_emb"], np.float32)[ids]
          + np.asarray(inputs["pos_emb"], np.float32)[None, :ids.shape[1], :]
          + np.asarray(inputs["type_emb"], np.float32)[tt])
    x0 = _np_layer_norm(x0, np.asarray(inputs["emb_ln_g"], np.float32),
                        np.asarray(inputs["emb_ln_b"], np.float32))
    mask = np.asarray(inputs["attention_mask"], np.float32)

    wqkv = np.asarray(inputs["wqkv"], np.float32)
    wfc_in = np.asarray(inputs["wfc"], np.float32)
    ln1_g = np.asarray(inputs["ln1_g"], np.float32)
    ln2_g = np.asarray(inputs["ln2_g"], np.float32)
    for name in ("bqkv", "bproj", "bfc", "bfc2", "ln1_b", "ln2_b"):
        assert np.abs(np.asarray(inputs[name])).max() == 0.0, (
            f"{name} is nonzero; this kernel folds only zero biases")
    wq_eff = wqkv * ln1_g[:, :, None]
    wf_eff = wfc_in * ln2_g[:, :, None]
    Lx = wqkv.shape[0]

    # interleave q/k per head pair: group g holds [q_pair_g | k_pair_g]
    q = wq_eff[:, :, 0 * D:1 * D].reshape(Lx, D, DK, P)
    k = wq_eff[:, :, 1 * D:2 * D].reshape(Lx, D, DK, P)
    qk_il = np.concatenate([q, k], axis=3)          # [L, D, 6, 256]
    qk_il = qk_il.reshape(Lx, D, 2 * D)
    wv_eff = np.ascontiguousarray(wq_eff[:, :, 2 * D:])

    wemb = np.asarray(inputs["word_emb"], np.float32)     # [V, D]
    wembT = np.zeros((D, VP), np.float32)
    wembT[:, :V] = wemb.T
    whd = _pack_km(wembT[None], NV)[0]                    # [60, 128, 6, 512]

    packed = dict(
        wqk=_pack_km(qk_il, 256).astype(NP_BF16),
        wv=np.ascontiguousarray(
            wv_eff.reshape(Lx, DK, P, D).transpose(0, 2, 1, 3)).astype(NP_BF16),
        csv=np.ascontiguousarray(wv_eff.sum(axis=1)[:, None, :]).astype(NP_BF16),
        wpr=np.ascontiguousarray(
            np.asarray(inputs["wproj"], np.float32)
            .reshape(Lx, DK, P, D).transpose(0, 2, 1, 3)).astype(NP_BF16),
        wfc=_pack_km(wf_eff, 512).astype(NP_BF16),
        wf2=_pack_km(np.asarray(inputs["wfc2"], np.float32), 128).astype(NP_BF16),
        whd=whd.astype(NP_BF16),
    )
    in_maps = []
    for b in range(B):
        bias = -10000.0 * (1.0 - mask[b])
        m = dict(packed)
        m["x0T"] = np.ascontiguousarray(x0[b].T).astype(np.float32)
        m["mb"] = np.ascontiguousarray(bias.reshape(TCH, P).T).astype(np.float32)
        in_maps.append(m)
    return in_maps


_NC_CACHE = {}


def get_nc():
    if "nc" not in _NC_CACHE:
        _NC_CACHE["nc"] = build(n_layers=12, with_head=True)
    return _NC_CACHE["nc"]


def kernel(**inputs) -> np.ndarray:
    nc = get_nc()
    in_maps = _prep_in_maps(inputs)
    res = bass_utils.run_bass_kernel_spmd(nc, in_maps, core_ids=list(range(NCORES)))
    return np.stack([res.results[b]["out"] for b in range(B)]).astype(np.float32)
